# revision 1
# baseline (speedup 1.0000x reference)
"""Trainium2 Bass kernel for a GCN-based DQN forward pass (8 NeuronCores).

Strategy: partition nodes+edges by destination node across 8 cores.
 - pass 1 (slot layout): edge MLP -> w, per-window reduce -> deg, dis=1/sqrt(deg+1)
 - table: per-core shard of dis * (x @ W_gcn) in bf16, AllGather -> full table
 - pass 2 (block layout): dma_gather source rows from the table, one-hot
   scatter matrices via a single tensor_scalar per 128-edge block, PSUM
   matmul accumulation per 128-node window, then pooling matmuls
 - AllReduce pooled sums/counts, replicated tiny MLP head
"""
import numpy as np
import ml_dtypes

BF16 = ml_dtypes.bfloat16


def _default_cfg():
    return dict(N=50000, E=1600000, G=64, A=8, NCORES=8, WIN=49, GRP=7)


def _derived(cfg):
    c = dict(cfg)
    c["SH_REAL"] = -(-c["N"] // c["NCORES"])          # real nodes per core (ceil)
    c["SH"] = c["WIN"] * 128                          # padded nodes per core
    assert c["SH"] >= c["SH_REAL"]
    c["NTOT"] = c["NCORES"] * c["SH"]
    assert c["NTOT"] % 2 == 0
    c["HALF"] = c["NTOT"] // 2
    assert c["HALF"] - 1 <= 32767, "half-table must be int16-indexable"
    c["NGRP"] = -(-c["WIN"] // c["GRP"])              # gather call groups
    return c


def _prep(cfg, x, edge_attr, W_e1, b_e1, W_e2, b_e2, W_gcn, b_gcn, W2, b2, W3, b3,
          edge_index, batch):
    """Host-side sharding/layout. Returns (in_maps, meta)."""
    N, E, G, A = cfg["N"], cfg["E"], cfg["G"], cfg["A"]
    NC, WIN, SH_REAL, SH = cfg["NCORES"], cfg["WIN"], cfg["SH_REAL"], cfg["SH"]
    HALF, GRP, NGRP = cfg["HALF"], cfg["GRP"], cfg["NGRP"]

    x = np.asarray(x, np.float32)
    edge_attr = np.asarray(edge_attr, np.float32)
    edge_index = np.asarray(edge_index)
    batch = np.asarray(batch)
    src = np.asarray(edge_index[0], np.int64)
    dst = np.asarray(edge_index[1], np.int64)
    attr = edge_attr[:, 0]

    deg = np.bincount(dst, minlength=N)

    # per-core degree-sorted window/slot assignment
    node_of_rank = np.full((NC, SH), -1, np.int64)   # rank -> orig node id (-1 pad)
    rank_of_orig = np.empty(N, np.int64)             # orig -> rank within its core
    R1_cw = np.zeros((NC, WIN), np.int64)
    for c in range(NC):
        lo, hi = c * SH_REAL, min((c + 1) * SH_REAL, N)
        nreal = hi - lo
        d_loc = np.full(SH, -1, np.int64)
        d_loc[:nreal] = deg[lo:hi]
        order = np.argsort(-d_loc, kind="stable")    # rank -> padded-loc
        rank = np.empty(SH, np.int64)
        rank[order] = np.arange(SH)
        node_of_rank[c] = np.where(order < nreal, lo + order, -1)
        rank_of_orig[lo:hi] = rank[:nreal]
        R1_cw[c] = np.maximum(d_loc[order].reshape(WIN, 128), 0).max(axis=1)

    R1_w = R1_cw.max(axis=0)
    R1TOT = max(int(R1_w.sum()), 1)
    woff1 = np.zeros(WIN + 1, np.int64)
    woff1[1:] = np.cumsum(R1_w)

    core_of = np.minimum(np.arange(N) // SH_REAL, NC - 1)
    gid_of_orig = core_of * SH + rank_of_orig        # global slot id

    # per-edge coordinates
    ecore = np.minimum(dst // SH_REAL, NC - 1)
    erank = rank_of_orig[dst]
    ew = erank // 128
    ep = erank % 128
    egid_src = gid_of_orig[src]
    ehalf = (egid_src >= HALF).astype(np.int64)

    # j = rank of edge within its dst-node's list (pass 1)
    eorder = np.argsort(dst, kind="stable")
    starts = np.zeros(N + 1, np.int64)
    starts[1:] = np.cumsum(deg)
    j1 = np.empty(E, np.int64)
    j1[eorder] = np.arange(E) - starts[dst[eorder]]

    # pass-2 segment = (window, half); per-core counts -> uniform block counts
    segid = ew * 2 + ehalf                            # 0..2*WIN-1
    cnt = np.zeros((NC, 2 * WIN), np.int64)
    for c in range(NC):
        m = ecore == c
        cnt[c] = np.bincount(segid[m], minlength=2 * WIN)
    NB_seg = -(-cnt.max(axis=0) // 128)               # blocks per segment (uniform)

    seg_boff = np.zeros(2 * WIN, np.int64)            # block offset of each segment
    MAXBLK = cfg.get("MAXBLK", 24)                    # max blocks per dma_gather call
    calls = []                                        # (half, block_start, nblocks)
    pos = 0
    # stream order: window-major, [half-0 blocks | half-1 blocks] per window, so
    # consumption order == call order (no buffer-slot deadlock)
    for w in range(WIN):
        for h in (0, 1):
            seg_boff[w * 2 + h] = pos
            nseg = int(NB_seg[w * 2 + h])
            b = pos
            pos += nseg
            while b < pos:
                nb = min(MAXBLK, pos - b)
                calls.append((h, int(b), int(nb)))
                b += nb
    NBLK = max(int(pos), 1)

    # per-window block list (block idx, is_first, is_last) in call order
    win_blocks = []
    for w in range(WIN):
        blks = []
        for h in (0, 1):
            b0 = int(seg_boff[w * 2 + h])
            blks.extend(range(b0, b0 + int(NB_seg[w * 2 + h])))
        win_blocks.append(blks)

    # j2 = rank of edge within its (core, segment) group
    keys = (ecore * (2 * WIN) + segid)
    eorder2 = np.argsort(keys, kind="stable")
    gcnt = np.bincount(keys, minlength=NC * 2 * WIN)
    gstarts = np.zeros(NC * 2 * WIN + 1, np.int64)
    gstarts[1:] = np.cumsum(gcnt)
    j2 = np.empty(E, np.int64)
    j2[eorder2] = np.arange(E) - gstarts[keys[eorder2]]

    ec_pack = np.stack([
        np.asarray(W_e1, np.float32).reshape(3),
        np.concatenate([np.asarray(b_e1, np.float32).reshape(1),
                        np.asarray(W_e2, np.float32).reshape(1),
                        np.asarray(b_e2, np.float32).reshape(1)]),
    ])  # not used; packed below properly

    we1 = np.asarray(W_e1, np.float32).reshape(3)
    ecv = np.array([we1[0], we1[1], we1[2],
                    np.asarray(b_e1, np.float32).reshape(-1)[0],
                    np.asarray(W_e2, np.float32).reshape(-1)[0],
                    np.asarray(b_e2, np.float32).reshape(-1)[0]], np.float32)
    ec_bcast = np.ascontiguousarray(np.broadcast_to(ecv, (128, 6)))

    iota128 = np.ascontiguousarray(
        np.broadcast_to(np.arange(128, dtype=np.float32), (128, 128)).astype(BF16))
    bgcn_b = np.ascontiguousarray(
        np.broadcast_to(np.asarray(b_gcn, np.float32), (128, 128)))
    b3_b = np.ascontiguousarray(
        np.broadcast_to(np.asarray(b3, np.float32), (64, A)))
    ident64 = np.eye(64, dtype=BF16)
    wgcn_np = np.ascontiguousarray(np.asarray(W_gcn, np.float32))
    w2_np = np.ascontiguousarray(np.asarray(W2, np.float32))
    w3_np = np.ascontiguousarray(np.asarray(W3, np.float32))
    b2_np = np.ascontiguousarray(np.asarray(b2, np.float32).reshape(128, 1))

    in_maps = []
    for c in range(NC):
        m = ecore == c
        s_src, s_dst, s_attr = src[m], dst[m], attr[m]
        s_ep, s_ew, s_j1, s_j2 = ep[m], ew[m], j1[m], j2[m]
        s_seg, s_half, s_gid = segid[m], ehalf[m], egid_src[m]

        # pass-1 slot-layout streams [128, R1TOT]
        p1_src = np.zeros((128, R1TOT), np.float32)
        p1_dst = np.zeros((128, R1TOT), np.float32)
        p1_attr = np.zeros((128, R1TOT), np.float32)
        p1_mask = np.zeros((128, R1TOT), np.float32)
        col1 = woff1[s_ew] + s_j1
        p1_src[s_ep, col1] = s_src
        p1_dst[s_ep, col1] = s_dst
        p1_attr[s_ep, col1] = s_attr
        p1_mask[s_ep, col1] = 1.0

        # pass-2 block-layout streams [128, NBLK]
        p2_src = np.zeros((128, NBLK), np.float32)
        p2_dst = np.zeros((128, NBLK), np.float32)
        p2_attr = np.zeros((128, NBLK), np.float32)
        p2_mask = np.zeros((128, NBLK), np.float32)
        p2_dstloc = np.zeros((128, NBLK), np.float32)
        blk = seg_boff[s_seg] + s_j2 // 128
        pp = s_j2 % 128
        p2_src[pp, blk] = s_src
        p2_dst[pp, blk] = s_dst
        p2_attr[pp, blk] = s_attr
        p2_mask[pp, blk] = 1.0
        p2_dstloc[pp, blk] = s_ep

        # gather idx stream, wrapped int16 [128, NBLK*8]
        idx_flat = np.zeros(NBLK * 128, np.int64)
        k = blk * 128 + pp
        idx_flat[k] = s_gid - s_half * HALF
        idx16 = np.zeros((128, NBLK * 8), np.int16)
        wrap = idx_flat.reshape(NBLK * 8, 16).T.astype(np.int16)
        for gg in range(8):
            idx16[gg * 16:(gg + 1) * 16, :] = wrap

        # xT in slot order [128, SH]
        nr = node_of_rank[c]
        xT = np.zeros((128, SH), np.float32)
        valid = nr >= 0
        xs = np.zeros((SH, x.shape[1]), np.float32)
        xs[valid] = x[nr[valid]]
        xT[:, :] = xs.T

        batch_slot = np.full((128, WIN), 127.0, np.float32)
        nmask = np.zeros((128, WIN), BF16)
        bvals = np.full(SH, 127, np.int64)
        bvals[valid] = batch[nr[valid]]
        batch_slot[:, :] = bvals.reshape(WIN, 128).T
        nmask[:, :] = (valid.reshape(WIN, 128).T).astype(BF16)

        in_maps.append({
            "p1_src": p1_src, "p1_dst": p1_dst, "p1_attr": p1_attr, "p1_mask": p1_mask,
            "p2_src": p2_src, "p2_dst": p2_dst, "p2_attr": p2_attr, "p2_mask": p2_mask,
            "p2_dstloc": p2_dstloc, "p2_idx": idx16,
            "xT": xT, "batch_slot": batch_slot, "nmask": nmask,
            "iota": iota128, "ecb": ec_bcast, "wgcn": wgcn_np, "bgcnb": bgcn_b,
            "w2": w2_np, "b2": b2_np, "w3": w3_np, "b3b": b3_b, "ident": ident64,
        })

    meta = dict(R1TOT=R1TOT, R1_w=[int(v) for v in R1_w],
                woff1=[int(v) for v in woff1],
                NBLK=NBLK, calls=calls, win_blocks=win_blocks)
    return in_maps, meta


def _build(cfg, meta, stop_at=None):
    from concourse import bass, bacc, tile
    import concourse.mybir as mybir

    do_table = stop_at not in ("pass1",)
    do_pass2 = stop_at not in ("pass1", "table")
    do_pool = stop_at not in ("pass1", "table", "pass2")
    do_head = stop_at is None

    f32 = mybir.dt.float32
    bf16 = mybir.dt.bfloat16
    i16 = mybir.dt.int16
    Alu = mybir.AluOpType
    Act = mybir.ActivationFunctionType

    NC, WIN, SH = cfg["NCORES"], cfg["WIN"], cfg["SH"]
    NTOT, HALF, G, A = cfg["NTOT"], cfg["HALF"], cfg["G"], cfg["A"]
    R1TOT, R1_w, woff1 = meta["R1TOT"], meta["R1_w"], meta["woff1"]
    NBLK, calls, win_blocks = meta["NBLK"], meta["calls"], meta["win_blocks"]

    nc = bacc.Bacc("TRN2", target_bir_lowering=False, debug=False, num_devices=NC)

    dram = lambda nm, shp, dt: nc.dram_tensor(nm, shp, dt, kind="ExternalInput")
    p1_src_d = dram("p1_src", [128, R1TOT], f32)
    p1_dst_d = dram("p1_dst", [128, R1TOT], f32)
    p1_attr_d = dram("p1_attr", [128, R1TOT], f32)
    p1_mask_d = dram("p1_mask", [128, R1TOT], f32)
    p2_src_d = dram("p2_src", [128, NBLK], f32)
    p2_dst_d = dram("p2_dst", [128, NBLK], f32)
    p2_attr_d = dram("p2_attr", [128, NBLK], f32)
    p2_mask_d = dram("p2_mask", [128, NBLK], f32)
    p2_dstloc_d = dram("p2_dstloc", [128, NBLK], f32)
    p2_idx_d = dram("p2_idx", [128, NBLK * 8], i16)
    xT_d = dram("xT", [128, SH], f32)
    batch_d = dram("batch_slot", [128, WIN], f32)
    nmask_d = dram("nmask", [128, WIN], bf16)
    iota_d = dram("iota", [128, 128], bf16)
    ecb_d = dram("ecb", [128, 6], f32)
    wgcn_d = dram("wgcn", [128, 128], f32)
    bgcnb_d = dram("bgcnb", [128, 128], f32)
    w2_d = dram("w2", [128, 128], f32)
    b2_d = dram("b2", [128, 1], f32)
    w3_d = dram("w3", [128, A], f32)
    b3b_d = dram("b3b", [64, A], f32)
    ident_d = dram("ident", [64, 64], bf16)
    out_d = nc.dram_tensor("out", [64, A], f32, kind="ExternalOutput")

    tabsh_d = nc.dram_tensor("tabsh", [SH, 128], bf16)
    tab_d = nc.dram_tensor("tab", [NTOT, 128], bf16, addr_space="Shared")
    pool_in_d = nc.dram_tensor("pool_in", [64, 129], f32)
    pool_out_d = nc.dram_tensor("pool_out", [64, 129], f32, addr_space="Shared")

    groups = [list(range(NC))]

    def mlp(pool, src_t, dst_t, attr_t, mask_t, ec_t, n):
        t = pool.tile([128, n], f32, tag="mlp_t")
        w_t = pool.tile([128, n], f32, tag="mlp_w")
        nc.vector.tensor_scalar(out=t[:], in0=src_t[:], scalar1=ec_t[:, 0:1],
                                scalar2=None, op0=Alu.mult)
        nc.vector.scalar_tensor_tensor(out=t[:], in0=dst_t[:], scalar=ec_t[:, 1:2],
                                       in1=t[:], op0=Alu.mult, op1=Alu.add)
        nc.vector.scalar_tensor_tensor(out=t[:], in0=attr_t[:], scalar=ec_t[:, 2:3],
                                       in1=t[:], op0=Alu.mult, op1=Alu.add)
        nc.scalar.activation(out=t[:], in_=t[:], func=Act.Relu, bias=ec_t[:, 3:4],
                             scale=1.0)
        nc.scalar.activation(out=w_t[:], in_=t[:], func=Act.Sigmoid,
                             bias=ec_t[:, 5:6], scale=ec_t[:, 4:5])
        nc.vector.tensor_tensor(out=w_t[:], in0=w_t[:], in1=mask_t[:], op=Alu.mult)
        return w_t

    with tile.TileContext(nc) as tc:
        with (
            tc.tile_pool(name="const", bufs=1) as cpool,
            tc.tile_pool(name="work", bufs=1) as wpool,
            tc.tile_pool(name="mtile", bufs=6) as mpool,
            tc.tile_pool(name="small", bufs=3) as spool,
            tc.tile_pool(name="psA", bufs=2, space="PSUM") as psA,
            tc.tile_pool(name="psB", bufs=1, space="PSUM") as psB,
            tc.tile_pool(name="psC", bufs=1, space="PSUM") as psC,
            tc.tile_pool(name="dram", bufs=1, space="DRAM") as _dp,
        ):
            # ---- constants ----
            iota_t = cpool.tile([128, 128], bf16)
            ec_t = cpool.tile([128, 6], f32)
            bgcn_t = cpool.tile([128, 128], f32)
            nmask_t = cpool.tile([128, WIN], bf16)
            batch_t = cpool.tile([128, WIN], f32)
            nc.sync.dma_start(out=iota_t[:], in_=iota_d[:])
            nc.sync.dma_start(out=ec_t[:], in_=ecb_d[:])
            nc.sync.dma_start(out=bgcn_t[:], in_=bgcnb_d[:])
            nc.sync.dma_start(out=nmask_t[:], in_=nmask_d[:])
            nc.sync.dma_start(out=batch_t[:], in_=batch_d[:])

            # ---- pass 1: edge MLP + deg + dis ----
            with tc.tile_pool(name="p1", bufs=1) as p1:
                p1s = p1.tile([128, R1TOT], f32)
                p1d = p1.tile([128, R1TOT], f32)
                p1a = p1.tile([128, R1TOT], f32)
                p1m = p1.tile([128, R1TOT], f32)
                nc.sync.dma_start(out=p1s[:], in_=p1_src_d[:])
                nc.sync.dma_start(out=p1d[:], in_=p1_dst_d[:])
                nc.sync.dma_start(out=p1a[:], in_=p1_attr_d[:])
                nc.sync.dma_start(out=p1m[:], in_=p1_mask_d[:])
                w1_t = mlp(p1, p1s, p1d, p1a, p1m, ec_t, R1TOT)

                deg_t = wpool.tile([128, WIN], f32)
                for w in range(WIN):
                    if R1_w[w] > 0:
                        nc.vector.tensor_reduce(
                            out=deg_t[:, w:w + 1],
                            in_=w1_t[:, woff1[w]:woff1[w] + R1_w[w]],
                            axis=mybir.AxisListType.X, op=Alu.add)
                    else:
                        nc.vector.memset(deg_t[:, w:w + 1], 0.0)
                nc.vector.tensor_scalar(out=deg_t[:], in0=deg_t[:], scalar1=1.0,
                                        scalar2=None, op0=Alu.add)
                sq_t = wpool.tile([128, WIN], f32)
                nc.scalar.activation(out=sq_t[:], in_=deg_t[:], func=Act.Sqrt)
                dis_t = wpool.tile([128, WIN], f32)
                nc.vector.reciprocal(out=dis_t[:], in_=sq_t[:])

            # ---- table build + AllGather ----
            if not do_table:
                dummy = wpool.tile([64, A], f32)
                nc.vector.memset(dummy[:], 0.0)
                nc.vector.tensor_scalar(out=dummy[:, 0:1], in0=dis_t[0:64, 0:1],
                                        scalar1=1.0, scalar2=None, op0=Alu.mult)
                nc.sync.dma_start(out=out_d[:], in_=dummy[:])
            if do_table:
                wgcn_f = cpool.tile([128, 128], f32)
                wgcn_b = cpool.tile([128, 128], bf16)
                nc.sync.dma_start(out=wgcn_f[:], in_=wgcn_d[:])
                nc.vector.tensor_copy(out=wgcn_b[:], in_=wgcn_f[:])
                for w in range(WIN):
                    xt = spool.tile([128, 128], f32, tag="xt")
                    nc.sync.dma_start(out=xt[:], in_=xT_d[:, w * 128:(w + 1) * 128])
                    xtb = spool.tile([128, 128], bf16, tag="xtb")
                    nc.vector.tensor_copy(out=xtb[:], in_=xt[:])
                    ps = psA.tile([128, 128], f32, tag="mm")
                    nc.tensor.matmul(ps[:], xtb[:], wgcn_b[:], start=True, stop=True)
                    tb = spool.tile([128, 128], bf16, tag="tb")
                    nc.vector.tensor_scalar(out=tb[:], in0=ps[:], scalar1=dis_t[:, w:w + 1],
                                            scalar2=None, op0=Alu.mult)
                    nc.sync.dma_start(out=tabsh_d[w * 128:(w + 1) * 128, :], in_=tb[:])
                nc.gpsimd.collective_compute(
                    "AllGather", Alu.bypass, replica_groups=groups,
                    ins=[tabsh_d[:]], outs=[tab_d[:]])

            # ---- pass 2 MLP ----
            if do_pass2:
                p2dl = wpool.tile([128, NBLK], f32)
                nc.sync.dma_start(out=p2dl[:], in_=p2_dstloc_d[:])
                idx_t = wpool.tile([128, NBLK * 8], i16)
                nc.sync.dma_start(out=idx_t[:], in_=p2_idx_d[:])
            if do_pass2:
                with tc.tile_pool(name="p2", bufs=1) as p2:
                    p2s = p2.tile([128, NBLK], f32)
                    p2d = p2.tile([128, NBLK], f32)
                    p2a = p2.tile([128, NBLK], f32)
                    p2m = p2.tile([128, NBLK], f32)
                    nc.sync.dma_start(out=p2s[:], in_=p2_src_d[:])
                    nc.sync.dma_start(out=p2d[:], in_=p2_dst_d[:])
                    nc.sync.dma_start(out=p2a[:], in_=p2_attr_d[:])
                    nc.sync.dma_start(out=p2m[:], in_=p2_mask_d[:])
                    w2s = mlp(p2, p2s, p2d, p2a, p2m, ec_t, NBLK)

                    # ---- pass 2: gather + one-hot matmul per window ----
                    # block -> (call position) mapping
                    blk_tile = {}
                    h1_tiles = []
                    call_tiles = []
                    for (h, b0, nb) in calls:
                        mt = mpool.tile([128, nb, 128], bf16, tag="M")
                        call_tiles.append((mt, h, b0, nb))
                        nc.gpsimd.dma_gather(
                            out_ap=mt[:],
                            in_ap=tab_d[h * HALF:(h + 1) * HALF, :],
                            idxs_ap=idx_t[:, b0 * 8:(b0 + nb) * 8],
                            num_idxs=nb * 128,
                            num_idxs_reg=nb * 128,
                            elem_size=128,
                            single_packet=False,
                        )
                        for i in range(nb):
                            blk_tile[b0 + i] = (mt, i)

                    for w in range(WIN):
                        blks = win_blocks[w]
                        tbw = spool.tile([128, 128], bf16, tag="tbw")
                        nc.sync.dma_start(out=tbw[:], in_=tabsh_d[w * 128:(w + 1) * 128, :])
                        fin = spool.tile([128, 128], f32, tag="fin")
                        if blks:
                            psw = psA.tile([128, 128], f32, tag="mm")
                            for bi, b in enumerate(blks):
                                mt, i = blk_tile[b]
                                s_t = spool.tile([128, 128], bf16, tag="s")
                                nc.vector.tensor_scalar(
                                    out=s_t[:], in0=iota_t[:],
                                    scalar1=p2dl[:, b:b + 1], scalar2=w2s[:, b:b + 1],
                                    op0=Alu.is_equal, op1=Alu.mult)
                                nc.tensor.matmul(psw[:], s_t[:], mt[:, i, :],
                                                 start=(bi == 0), stop=(bi == len(blks) - 1))
                            nc.vector.tensor_tensor(out=fin[:], in0=psw[:], in1=tbw[:],
                                                    op=Alu.add)
                        else:
                            nc.vector.tensor_copy(out=fin[:], in_=tbw[:])
                        nc.vector.scalar_tensor_tensor(
                            out=fin[:], in0=fin[:], scalar=dis_t[:, w:w + 1],
                            in1=bgcn_t[:], op0=Alu.mult, op1=Alu.add)
                        h1 = wpool.tile([128, 129], bf16, tag=f"h1_{w}")
                        nc.scalar.activation(out=h1[:, 0:128], in_=fin[:], func=Act.Relu)
                        nc.vector.tensor_copy(out=h1[:, 128:129], in_=nmask_t[:, w:w + 1])
                        h1_tiles.append(h1)

                    # ---- pooling matmuls ----
                    pool_ps = psB.tile([64, 129], f32, tag="poolps", name="pool_ps") if do_pool else None
                    for w in range(WIN if do_pool else 0):
                        pw = spool.tile([128, 64], bf16, tag="pw")
                        nc.vector.tensor_scalar(
                            out=pw[:], in0=iota_t[:, 0:64],
                            scalar1=batch_t[:, w:w + 1], scalar2=None, op0=Alu.is_equal)
                        nc.tensor.matmul(pool_ps[:], pw[:], h1_tiles[w][:],
                                         start=(w == 0), stop=(w == WIN - 1))

            # ---- AllReduce pooled ----
            if do_pool:
                pool_sb = wpool.tile([64, 129], f32)
                nc.vector.tensor_copy(out=pool_sb[:], in_=pool_ps[:])
                nc.sync.dma_start(out=pool_in_d[:], in_=pool_sb[:])
                nc.gpsimd.collective_compute(
                    "AllReduce", Alu.add, replica_groups=groups,
                    ins=[pool_in_d[:]], outs=[pool_out_d[:]])
                pool2 = wpool.tile([64, 129], f32)
                nc.sync.dma_start(out=pool2[:], in_=pool_out_d[:])

            # ---- head ----
            if do_head:
                cntm = wpool.tile([64, 1], f32)
                nc.vector.tensor_scalar(out=cntm[:], in0=pool2[:, 128:129], scalar1=1.0,
                                        scalar2=None, op0=Alu.max)
                rec = wpool.tile([64, 1], f32)
                nc.vector.reciprocal(out=rec[:], in_=cntm[:])
                pooled_b = wpool.tile([64, 128], bf16)
                nc.vector.tensor_scalar(out=pooled_b[:], in0=pool2[:, 0:128],
                                        scalar1=rec[:], scalar2=None, op0=Alu.mult)

                ident_t = cpool.tile([64, 64], bf16)
                nc.sync.dma_start(out=ident_t[:], in_=ident_d[:])
                psT = psC.tile([128, 64], bf16, tag="pT")
                nc.tensor.transpose(psT[:], pooled_b[:], ident_t[:])
                pooledT = wpool.tile([128, 64], bf16)
                nc.vector.tensor_copy(out=pooledT[:], in_=psT[:])

                w2f = cpool.tile([128, 128], f32)
                w2b = cpool.tile([128, 128], bf16)
                nc.sync.dma_start(out=w2f[:], in_=w2_d[:])
                nc.vector.tensor_copy(out=w2b[:], in_=w2f[:])
                b2_t = cpool.tile([128, 1], f32)
                nc.sync.dma_start(out=b2_t[:], in_=b2_d[:])
                h2ps = psC.tile([128, 64], f32, tag="h2")
                nc.tensor.matmul(h2ps[:], w2b[:], pooledT[:], start=True, stop=True)
                h2sb = wpool.tile([128, 64], bf16)
                nc.scalar.activation(out=h2sb[:], in_=h2ps[:], func=Act.Relu,
                                     bias=b2_t[:], scale=1.0)

                w3f = cpool.tile([128, A], f32)
                w3b = cpool.tile([128, A], bf16)
                nc.sync.dma_start(out=w3f[:], in_=w3_d[:])
                nc.vector.tensor_copy(out=w3b[:], in_=w3f[:])
                b3_t = cpool.tile([64, A], f32)
                nc.sync.dma_start(out=b3_t[:], in_=b3b_d[:])
                yps = psC.tile([64, A], f32, tag="y")
                nc.tensor.matmul(yps[:], h2sb[:], w3b[:], start=True, stop=True)
                ysb = wpool.tile([64, A], f32)
                nc.vector.tensor_tensor(out=ysb[:], in0=yps[:], in1=b3_t[:], op=Alu.add)
                nc.sync.dma_start(out=out_d[:], in_=ysb[:])
            if do_table and not do_head:
                dummy2 = wpool.tile([64, A], f32)
                nc.vector.memset(dummy2[:], 0.0)
                nc.sync.dma_start(out=out_d[:], in_=dummy2[:])

    nc.compile()
    return nc


_CACHE = {}


def _get_program(cfg, meta):
    key = (tuple(sorted(cfg.items())), meta["R1TOT"], tuple(meta["R1_w"]),
           meta["NBLK"], tuple(meta["calls"]),
           tuple(tuple(b) for b in meta["win_blocks"]))
    if key not in _CACHE:
        _CACHE[key] = _build(cfg, meta)
    return _CACHE[key]


def kernel(**inputs):
    from concourse import bass_utils
    cfg = _derived(_default_cfg())
    inputs = {k: np.asarray(v) for k, v in inputs.items()}
    in_maps, meta = _prep(cfg, **inputs)
    nc = _get_program(cfg, meta)
    res = bass_utils.run_bass_kernel_spmd(nc, in_maps, list(range(cfg["NCORES"])))
    return np.asarray(res.results[0]["out"], np.float32)[: cfg["G"]]

